# revision 1
# baseline (speedup 1.0000x reference)
"""Bidirectional 2-layer GRU kernel for TRN2, data-parallel over batch (B=64 -> 8 cores x 8).

Per-core layout:
  - 4 chains: ci 0=f0, 1=b0, 2=f1, 3=b1 at PE col-tile / partition offsets {0,32,64,96}.
  - Recurrence matmuls (mapping A): out gh [batch(8) x 1536] per chain, stationary = hT
    chunk [128,8] (fp32r), moving = WhhT slices [128,512] (fp32r), col-tiled so the 4
    chains stream concurrently.
  - Elementwise in "spread" partition layout [0:104] covering all chains per instruction.
  - Layer-1 trails layer-0 by one chunk (C steps); gi GEMMs amortized per chunk in-core.
"""
import sys
sys.path.insert(0, "/opt/trn_rl_repo")
import numpy as np
import concourse.bass as bass
import concourse.bacc as bacc
import concourse.mybir as mybir
from concourse import tile
from concourse import bass_utils

F32 = mybir.dt.float32
F32R = mybir.dt.float32r
AT = mybir.ActivationFunctionType
OP = mybir.AluOpType

H = 512
TH = 1536
B = 8          # batch per core
POFF = [0, 32, 64, 96]


def build(S=512, C=16, n_cores=8):
    CH = S // C
    assert CH * C == S
    nc = bacc.Bacc("TRN2", target_bir_lowering=False, debug=False, num_devices=n_cores)

    # ---- DRAM I/O (per core) ----
    xT_f = nc.dram_tensor("xT_f", [4, 128, S, B], F32, kind="ExternalInput")
    xT_b = nc.dram_tensor("xT_b", [4, 128, S, B], F32, kind="ExternalInput")
    WhhT = nc.dram_tensor("WhhT", [4, 128, 4, TH], F32, kind="ExternalInput")
    WihT = nc.dram_tensor("WihT", [4, 4, 128, TH], F32, kind="ExternalInput")
    bias_gi = nc.dram_tensor("bias_gi", [4, TH], F32, kind="ExternalInput")
    bhh_n = nc.dram_tensor("bhh_n", [4, H], F32, kind="ExternalInput")
    h0_spread = nc.dram_tensor("h0_spread", [128, H], F32, kind="ExternalInput")
    h0T = nc.dram_tensor("h0T", [128, 128], F32, kind="ExternalInput")
    ones_in = nc.dram_tensor("ones_in", [4, 128], F32, kind="ExternalInput")
    ident_in = nc.dram_tensor("ident_in", [128, 128], F32, kind="ExternalInput")
    out_f = nc.dram_tensor("out_f", [B, S, H], F32, kind="ExternalOutput")
    out_b = nc.dram_tensor("out_b", [B, S, H], F32, kind="ExternalOutput")
    h_out = nc.dram_tensor("h_out", [B, 2 * H], F32, kind="ExternalOutput")

    with tile.TileContext(nc) as tc:
        import contextlib
        ctx = contextlib.ExitStack()
        with ctx:
            const = ctx.enter_context(tc.tile_pool(name="const", bufs=1))
            whh_p = ctx.enter_context(tc.tile_pool(name="whh", bufs=1))
            wih_p = ctx.enter_context(tc.tile_pool(name="wih", bufs=6))
            xt_p = ctx.enter_context(tc.tile_pool(name="xt", bufs=6))
            gi_ps = [ctx.enter_context(tc.tile_pool(name=f"gi{d}", bufs=2)) for d in range(4)]
            stg_ps = [ctx.enter_context(tc.tile_pool(name=f"stg{d}", bufs=2)) for d in range(2)]
            h_p = ctx.enter_context(tc.tile_pool(name="h", bufs=2))
            hT_p = ctx.enter_context(tc.tile_pool(name="hT", bufs=2))
            gisp_p = ctx.enter_context(tc.tile_pool(name="gisp", bufs=2))
            srz_p = ctx.enter_context(tc.tile_pool(name="srz", bufs=2))
            tn_p = ctx.enter_context(tc.tile_pool(name="tn", bufs=1))
            nn_p = ctx.enter_context(tc.tile_pool(name="nn", bufs=1))
            d_p = ctx.enter_context(tc.tile_pool(name="d", bufs=1))
            mm_p = ctx.enter_context(tc.tile_pool(name="mm", bufs=1, space="PSUM"))
            gp_p = ctx.enter_context(tc.tile_pool(name="gp", bufs=3, space="PSUM"))
            ptr_p = ctx.enter_context(tc.tile_pool(name="ptr", bufs=2, space="PSUM"))

            # ---- constants ----
            whh_sb = whh_p.tile([128, 4 * 4 * TH], F32R)
            for ci in range(4):
                nc.sync.dma_start(whh_sb[:, ci * 4 * TH:(ci + 1) * 4 * TH],
                                  WhhT.ap()[ci].bitcast(F32R))
            ones_sb = const.tile([4, 128], F32R)
            nc.sync.dma_start(ones_sb[:], ones_in.ap().bitcast(F32R))
            bias_gi_sb = const.tile([4, TH], F32R)
            nc.sync.dma_start(bias_gi_sb[:], bias_gi.ap().bitcast(F32R))
            bhh_n_sb = const.tile([4, H], F32R)
            nc.sync.dma_start(bhh_n_sb[:], bhh_n.ap().bitcast(F32R))
            ident_sb = const.tile([128, 128], F32)
            nc.sync.dma_start(ident_sb[:], ident_in.ap())
            h0_sb = const.tile([128, H], F32)
            nc.sync.dma_start(h0_sb[:], h0_spread.ap())
            h0T_sb = const.tile([128, 128], F32R)
            nc.sync.dma_start(h0T_sb[:], h0T.ap().bitcast(F32R))

            h_prev = h0_sb
            hT_prev = h0T_sb
            gi_cur = [None] * 4   # current SBUF gi tiles per dir-layer
            stg_prev = [None, None]  # staging tiles (f, b) filled during previous chunk
            stg_cur = [None, None]

            def gemm(dl, stat_aps, gi_tile):
                """gi_tile[:, :] = stationary.T @ WihT[dl] + bias (rows = C*B)."""
                wt = []
                for k in range(4):
                    t = wih_p.tile([128, TH], F32R, tag="wih")
                    nc.sync.dma_start(t[:], WihT.ap()[dl, k].bitcast(F32R))
                    wt.append(t)
                for n in range(3):
                    gp = gp_p.tile([128, 512], F32, tag="gp")
                    for k in range(4):
                        nc.tensor.matmul(gp[:, :], stat_aps[k], wt[k][:, n * 512:(n + 1) * 512],
                                         start=(k == 0), stop=False, skip_group_check=True)
                    nc.tensor.matmul(gp[:, :], ones_sb[dl:dl + 1, :],
                                     bias_gi_sb[dl:dl + 1, n * 512:(n + 1) * 512],
                                     start=False, stop=True, skip_group_check=True)
                    if n % 2 == 0:
                        nc.scalar.copy(gi_tile[:, n * 512:(n + 1) * 512], gp[:, :])
                    else:
                        nc.vector.tensor_copy(gi_tile[:, n * 512:(n + 1) * 512], gp[:, :])

            for c in range(CH + 1):
                l0 = c < CH
                l1 = c >= 1
                # -- chunk-level GEMMs --
                if l0:
                    for dl, xT in ((0, xT_f), (1, xT_b)):
                        xts = []
                        for k in range(4):
                            t = xt_p.tile([128, C * B], F32R, tag="xt")
                            nc.sync.dma_start(t[:], xT.ap()[k, :, c * C:(c + 1) * C, :].bitcast(F32R))
                            xts.append(t[:])
                        gi_cur[dl] = gi_ps[dl].tile([128, TH], F32, tag=f"gi{dl}")
                        gemm(dl, xts, gi_cur[dl])
                    stg_prev[0], stg_prev[1] = stg_cur[0], stg_cur[1]
                    stg_cur[0] = stg_ps[0].tile([128, 4, C, B], F32R, tag="stg0")
                    stg_cur[1] = stg_ps[1].tile([128, 4, C, B], F32R, tag="stg1")
                else:
                    stg_prev[0], stg_prev[1] = stg_cur[0], stg_cur[1]
                if l1:
                    for dl in (2, 3):
                        stg = stg_prev[dl - 2]
                        stats = [stg[:, k, :, :] for k in range(4)]
                        gi_cur[dl] = gi_ps[dl].tile([128, TH], F32, tag=f"gi{dl}")
                        gemm(dl, stats, gi_cur[dl])

                if l0 and l1:
                    active = [0, 1, 2, 3]
                    sp = slice(0, 104)
                    tr_base = 0
                    cpairs = [(0, 0), (1, 1), (2, 2), (3, 3)]
                elif l0:
                    active = [0, 1]
                    sp = slice(0, 40)
                    tr_base = 0
                    cpairs = [(0, 0), (1, 1)]
                else:
                    active = [2, 3]
                    sp = slice(64, 104)
                    tr_base = 64
                    cpairs = [(0, 2), (1, 3)]

                # -- ticks --
                for i in range(C):
                    tau = c * C + i
                    sigma = tau - C
                    # gather gi slices into spread tile
                    gisp = gisp_p.tile([128, TH], F32, tag="gisp")
                    for ci in active:
                        step = tau if ci < 2 else sigma
                        r0 = (step % C) * B
                        nc.sync.dma_start(gisp[POFF[ci]:POFF[ci] + B, :],
                                          gi_cur[ci][r0:r0 + B, :])
                    # hidden matmuls
                    mm = mm_p.tile([128, TH], F32, tag="mm")
                    for n in range(3):
                        first = True
                        for k in range(4):
                            for ci in active:
                                off = POFF[ci]
                                nc.tensor.matmul(
                                    mm[off:off + B, n * 512:(n + 1) * 512],
                                    hT_prev[:, k * 32 + ci * 8: k * 32 + ci * 8 + B],
                                    whh_sb[:, (ci * 4 + k) * TH + n * 512: (ci * 4 + k) * TH + (n + 1) * 512],
                                    start=first, stop=(n < 2 and k == 3),
                                    tile_position=(0, off), skip_group_check=True)
                                first = False
                    for ci in active:
                        off = POFF[ci]
                        nc.tensor.matmul(mm[off:off + B, 1024:1536],
                                         ones_sb[ci:ci + 1, off:off + B],
                                         bhh_n_sb[ci:ci + 1, :],
                                         start=False, stop=True,
                                         tile_position=(0, off), skip_group_check=True)
                    # elementwise (spread)
                    srz = srz_p.tile([128, 1024], F32, tag="srz")
                    nc.vector.tensor_tensor(srz[sp, :], mm[sp, 0:1024], gisp[sp, 0:1024], OP.add)
                    nc.scalar.activation(srz[sp, :], srz[sp, :], AT.Sigmoid)
                    tn = tn_p.tile([128, H], F32, tag="tn")
                    nc.vector.scalar_tensor_tensor(tn[sp, :], mm[sp, 1024:1536], 1.0,
                                                   srz[sp, 0:512], OP.mult, OP.mult)
                    nc.vector.tensor_tensor(tn[sp, :], tn[sp, :], gisp[sp, 1024:1536], OP.add)
                    nn = nn_p.tile([128, H], F32, tag="nn")
                    nc.scalar.activation(nn[sp, :], tn[sp, :], AT.Tanh)
                    d = d_p.tile([128, H], F32, tag="d")
                    nc.gpsimd.scalar_tensor_tensor(d[sp, :], h_prev[sp, :], 1.0, nn[sp, :],
                                                   OP.mult, OP.subtract)
                    nc.gpsimd.scalar_tensor_tensor(d[sp, :], srz[sp, 512:1024], 1.0, d[sp, :],
                                                   OP.mult, OP.mult)
                    h_new = h_p.tile([128, H], F32, tag="h")
                    nc.vector.tensor_tensor(h_new[sp, :], nn[sp, :], d[sp, :], OP.add)
                    # transposes -> hT
                    ptr = ptr_p.tile([128, 4, 128], F32, tag="ptr")
                    nsp = sp.stop - sp.start
                    for j in range(4):
                        nc.tensor.transpose(ptr[:, j, 0:nsp],
                                            h_new[sp, j * 128:(j + 1) * 128],
                                            ident_sb[0:nsp, 0:nsp])
                    hT_new = hT_p.tile([128, 128], F32R, tag="hT")
                    ptr_r = ptr[:].rearrange("p j (c w) -> p j c w", c=4)
                    hT_r = hT_new[:].rearrange("p (j c b) -> p j c b", j=4, c=4)
                    for (sc, dc) in cpairs:
                        pass
                    src_cs = [s for (s, _) in cpairs]
                    dst_cs = [dd for (_, dd) in cpairs]
                    if dst_cs == [0, 1, 2, 3]:
                        nc.scalar.copy(hT_r[:, :, :, 0:B], ptr_r[:, :, :, 0:B])
                    elif dst_cs == [0, 1]:
                        nc.scalar.copy(hT_r[:, :, 0:2, 0:B], ptr_r[:, :, 0:2, 0:B])
                    else:  # tail: src c 0,1 -> dst c 2,3
                        nc.scalar.copy(hT_r[:, :, 2:4, 0:B], ptr_r[:, :, 0:2, 0:B])
                    # staging copies (L0 only)
                    if l0:
                        nc.scalar.copy(stg_cur[0][:, :, i, :], ptr_r[:, :, 0, 0:B])
                        nc.scalar.copy(stg_cur[1][:, :, i, :], ptr_r[:, :, 1, 0:B])
                    # chunk-0 patch: seed L1 state with h0 for chunk 1
                    if c == 0 and i == C - 1:
                        h0T_r = h0T_sb[:].rearrange("p (j c b) -> p j c b", j=4, c=4)
                        nc.scalar.copy(hT_r[:, :, 2:4, :], h0T_r[:, :, 2:4, :])
                        nc.scalar.copy(h_new[64:104, :], h0_sb[64:104, :])
                    # output writes (L1)
                    if l1:
                        nc.sync.dma_start(out_f.ap()[:, sigma, :], h_new[64:64 + B, :])
                        nc.sync.dma_start(out_b.ap()[:, S - 1 - sigma, :], h_new[96:96 + B, :])
                        if sigma == S - 1:
                            nc.sync.dma_start(h_out.ap()[:, 0:H], h_new[64:64 + B, :])
                            nc.sync.dma_start(h_out.ap()[:, H:2 * H], h_new[96:96 + B, :])
                    h_prev = h_new
                    hT_prev = hT_new
    nc.compile()
    return nc


# ---------------- host-side helpers ----------------

def prep_core_inputs(x_c, enc_c, Wih_f, Whh_f, bih_f, bhh_f, Wih_b, Whh_b, bih_b, bhh_b, S):
    """x_c: [B, S, I] slice for this core; enc_c: [B, 2H]."""
    I = x_c.shape[2]
    t = np.ascontiguousarray(x_c.transpose(2, 1, 0))          # [I, S, B]
    xT_f = t.reshape(4, 128, S, B)
    tb = np.ascontiguousarray(x_c[:, ::-1, :].transpose(2, 1, 0))
    xT_b = tb.reshape(4, 128, S, B)

    # chains: f0, b0, f1, b1
    Whh = [Whh_f[0], Whh_b[0], Whh_f[1], Whh_b[1]]
    Wih = [Wih_f[0], Wih_b[0], Wih_f[1], Wih_b[1]]
    bih = [bih_f[0], bih_b[0], bih_f[1], bih_b[1]]
    bhh = [bhh_f[0], bhh_b[0], bhh_f[1], bhh_b[1]]

    WhhT = np.stack([np.ascontiguousarray(W.T).reshape(4, 128, TH).transpose(1, 0, 2)
                     for W in Whh])                            # [4, 128, 4, TH]
    WihT = np.stack([np.ascontiguousarray(W.T).reshape(4, 128, TH) for W in Wih])  # [4,4,128,TH]
    bias_gi = np.stack([bih[i] + np.concatenate([bhh[i][:1024], np.zeros(512, np.float32)])
                        for i in range(4)]).astype(np.float32)  # [4, TH]
    bhh_n = np.stack([bhh[i][1024:1536] for i in range(4)]).astype(np.float32)  # [4, H]

    h0f = enc_c[:, :H]
    h0b = enc_c[:, H:]
    h0_spread = np.zeros((128, H), np.float32)
    for ci, h0 in enumerate([h0f, h0b, h0f, h0b]):
        h0_spread[POFF[ci]:POFF[ci] + B] = h0
    h0T = np.zeros((128, 128), np.float32)
    for ci, h0 in enumerate([h0f, h0b, h0f, h0b]):
        for j in range(4):
            h0T[:, j * 32 + ci * 8: j * 32 + ci * 8 + B] = h0[:, j * 128:(j + 1) * 128].T
    ones = np.ones((4, 128), np.float32)
    ident = np.eye(128, dtype=np.float32)
    return {
        "xT_f": np.ascontiguousarray(xT_f), "xT_b": np.ascontiguousarray(xT_b),
        "WhhT": np.ascontiguousarray(WhhT.astype(np.float32)),
        "WihT": np.ascontiguousarray(WihT.astype(np.float32)),
        "bias_gi": bias_gi, "bhh_n": bhh_n,
        "h0_spread": h0_spread, "h0T": h0T, "ones_in": ones, "ident_in": ident,
    }


def run_full(nc, inputs, S, n_cores=8, trace=False, **kw):
    """inputs: the full problem dict from setup_inputs (numpy). Returns (out, h, results)."""
    x = np.asarray(inputs["x"])
    enc = np.asarray(inputs["encoder_h"])
    Bfull = x.shape[0]
    bloc = Bfull // n_cores
    assert bloc == B
    in_maps = []
    for cidx in range(n_cores):
        sl = slice(cidx * B, (cidx + 1) * B)
        in_maps.append(prep_core_inputs(
            x[sl], enc[sl],
            np.asarray(inputs["Wih_f"]), np.asarray(inputs["Whh_f"]),
            np.asarray(inputs["bih_f"]), np.asarray(inputs["bhh_f"]),
            np.asarray(inputs["Wih_b"]), np.asarray(inputs["Whh_b"]),
            np.asarray(inputs["bih_b"]), np.asarray(inputs["bhh_b"]), S))
    res = bass_utils.run_bass_kernel_spmd(nc, in_maps, core_ids=list(range(n_cores)),
                                          trace=trace, **kw)
    outs = []
    hs = []
    for cidx in range(n_cores):
        r = res.results[cidx]
        outs.append(np.concatenate([r["out_f"], r["out_b"]], axis=2))
        hs.append(r["h_out"])
    out = np.concatenate(outs, axis=0)
    h = np.concatenate(hs, axis=0)
    return out, h, res


_NC_CACHE = {}


def _get_nc():
    if "nc" not in _NC_CACHE:
        _NC_CACHE["nc"] = build(S=512, C=16, n_cores=8)
    return _NC_CACHE["nc"]


def kernel(x, encoder_h, Wih_f, Whh_f, bih_f, bhh_f, Wih_b, Whh_b, bih_b, bhh_b):
    """Full-input entry point: shards batch over 8 NeuronCores, returns (out, h)."""
    nc = _get_nc()
    inputs = {
        "x": np.asarray(x, np.float32), "encoder_h": np.asarray(encoder_h, np.float32),
        "Wih_f": np.asarray(Wih_f, np.float32), "Whh_f": np.asarray(Whh_f, np.float32),
        "bih_f": np.asarray(bih_f, np.float32), "bhh_f": np.asarray(bhh_f, np.float32),
        "Wih_b": np.asarray(Wih_b, np.float32), "Whh_b": np.asarray(Whh_b, np.float32),
        "bih_b": np.asarray(bih_b, np.float32), "bhh_b": np.asarray(bhh_b, np.float32),
    }
    out, h, _ = run_full(nc, inputs, 512, n_cores=8)
    return out, h
